# revision 22
# baseline (speedup 1.0000x reference)
"""CapsNet (nn_CapsNet_25194278158439) Trainium2 Bass kernel, 8-core SPMD.

Math (faithful to the reference, including its W-tiling quirk):
  conv1 (9x9 valid) + relu -> conv2 (9x9 stride2) + bias -> primary capsules
  prim[b, p, d],  p = t*576 + s  (t = capsule type 0..31, s = spatial 0..575)
  u_hat[b,p,c,:] = prim[b,p,:] @ W[p mod 32]   (jnp.tile => weight index = s mod 32)
  3 rounds of routing-by-agreement over C=276 classes; out = ||v||  [2, 276].

Key factorization: u_hat is never materialized.  With g = s mod 32,
  s_vec[b,c,e]  = sum_g sum_d m[b,g,c,d] * W[g,c,d,e],
  m[b,g,c,d]    = sum_{p in g} softmax_c(b_log)[b,p,c] * prim[b,p,d]   (matmul)
  b_log update  = prim @ (W[g] @ v)                                    (matmul)

Sharding (8 cores):
  conv2 partial units (b, oc_half, ic_half).  Partials are exchanged with a
  single AllToAll of 294KB per core (each core receives exactly its 4 weight
  groups' columns from every rank, at static offsets -- no register-dependent
  DMAs).  ic-half partials are summed on the tensor engine during the
  stage->h2T transposes (PSUM accumulate).  Each routing iteration does one
  48KB AllReduce of the class vote s.
"""

import os
import numpy as np
import ml_dtypes

BF16 = ml_dtypes.bfloat16

NC = 8          # cores
C = 276         # classes
D = 8           # primary capsule dim
E = 16          # digit capsule dim
NT = 32         # capsule types
S = 576         # spatial positions per type (24*24)
GL = 4          # weight groups per core
Q = 8           # row blocks per core: (batch, local group)
PCH = 5         # 128-row chunks per 640-padded block
CP = 384        # class dim padded to 3*128
CCH = 3         # class chunks

_CACHE = {}


def _build_program():
    import concourse.bass as bass
    import concourse.mybir as mybir
    import concourse.tile as tile
    from concourse import bacc
    from concourse.bass import ds
    from concourse.masks import make_identity

    f32 = mybir.dt.float32
    f32r = mybir.dt.float32r
    AX = mybir.AxisListType
    AF = mybir.ActivationFunctionType
    ALU = mybir.AluOpType

    nc = bacc.Bacc("TRN2", target_bir_lowering=False, debug=False,
                   num_devices=NC)

    # ---- kernel I/O -------------------------------------------------------
    bf16 = mybir.dt.bfloat16
    xb = nc.dram_tensor("xb", [64, 64], bf16, kind="ExternalInput").ap()
    w1T = nc.dram_tensor("w1T", [81, 128], bf16, kind="ExternalInput").ap()
    b1 = nc.dram_tensor("b1", [128, 1], f32, kind="ExternalInput").ap()
    w2T = nc.dram_tensor("w2T", [128, 81 * 128], bf16,
                         kind="ExternalInput").ap()
    biasT = nc.dram_tensor("biasT", [128, PCH * D], bf16,
                           kind="ExternalInput").ap()
    Wcf = nc.dram_tensor("Wcf", [128, CCH * E * GL * D], bf16,
                         kind="ExternalInput").ap()
    Wcf2 = nc.dram_tensor("Wcf2", [128, CCH * GL * D * E], bf16,
                          kind="ExternalInput").ap()
    out = nc.dram_tensor("out", [2, C], f32, kind="ExternalOutput").ap()

    with tile.TileContext(nc) as tc:
        import contextlib
        with contextlib.ExitStack() as ctx:
            ctx.enter_context(nc.allow_low_precision(
                reason="tolerance 2e-2; bf16 path validated vs reference"))
            pool = ctx.enter_context(tc.tile_pool(name="const", bufs=1))
            dram = ctx.enter_context(tc.tile_pool(name="dram", bufs=1,
                                                  space="DRAM"))

            ident = pool.tile([128, 128], f32, tag="ident")
            make_identity(nc, ident[:])
            ident_bf = pool.tile([128, 128], bf16, tag="ident_bf")
            nc.vector.tensor_copy(ident_bf[:], ident[:])
            epsc = pool.tile([128, 1], f32, tag="epsc")
            zeroc = pool.tile([128, 1], f32, tag="zeroc")
            nc.vector.memset(epsc[:], 1e-8)
            nc.vector.memset(zeroc[:], 0.0)

            # collective bounce buffers
            a2a_in = dram.tile([1024, 72], bf16, tag="a2a_in")
            a2a_out = dram.tile([1024, 72], bf16, tag="a2a_out")
            s_ins = [dram.tile([128, 96], f32, tag=f"s_in{i}",
                                name=f"s_in{i}") for i in range(3)]
            s_outs = [dram.tile([128, 96], f32, tag=f"s_out{i}",
                                name=f"s_out{i}",
                                addr_space="Shared") for i in range(3)]

            # ============ conv phase ======================================
            with contextlib.ExitStack() as cctx:
                cpool = cctx.enter_context(tc.tile_pool(name="conv", bufs=1))
                ps1 = cctx.enter_context(
                    tc.tile_pool(name="ps1", bufs=2, space="PSUM"))
                ps2 = cctx.enter_context(
                    tc.tile_pool(name="ps2", bufs=1, space="PSUM"))

                w1T_sb = cpool.tile([81, 128], bf16, tag="w1T")
                b1_sb = cpool.tile([128, 1], f32, tag="b1")
                patches = cpool.tile([81, 3136], bf16, tag="patches")
                h1 = cpool.tile([128, 3136], bf16, tag="h1")
                w2T_sb = cpool.tile([128, 81 * 128], bf16, tag="w2T")
                h2p3 = cpool.tile([128, 576], bf16, tag="h2p3")

                nc.scalar.dma_start(w1T_sb[:], w1T)
                nc.scalar.dma_start(b1_sb[:], b1)

                # conv1 im2col: patches[(kh,kw), (oh,ow)] = x[oh+kh, ow+kw]
                for kh in range(9):
                    src = bass.AP(tensor=xb.tensor, offset=kh * 64,
                                  ap=[[1, 9], [64, 56], [1, 56]])
                    nc.sync.dma_start(
                        patches[kh * 9:(kh + 1) * 9, :].rearrange(
                            "p (a b) -> p a b", a=56), src)

                # conv2 weights: chunked DMAs on the sync DGE
                for ci in range(4):
                    nc.sync.dma_start(
                        w2T_sb[:, ci * 2592: (ci + 1) * 2592],
                        w2T[:, ci * 2592: (ci + 1) * 2592])

                # conv1: h1[oc, s] = relu(w1.T @ patches + b1)
                for j in range(7):
                    pt = ps1.tile([128, 448], f32, tag="c1")
                    nc.tensor.matmul(pt[:], w1T_sb[:],
                                     patches[:, j * 448:(j + 1) * 448],
                                     start=True, stop=True)
                    nc.scalar.activation(h1[:, j * 448:(j + 1) * 448], pt[:],
                                         AF.Relu, bias=b1_sb[:, 0:1])

                # conv2: 81-position accumulation, stride 2
                psA = ps2.tile([128, 288], f32, tag="psA")
                psB = ps2.tile([128, 288], f32, tag="psB")
                hv = h1[:].rearrange("p (h w) -> p h w", w=56)
                for pos in range(81):
                    kh, kw = divmod(pos, 9)
                    vh = hv.rearrange("p (oh two) w -> p oh two w", two=2)[
                        :, kh // 2: kh // 2 + 24, kh % 2, :]
                    vw = vh.rearrange("p oh (ow two) -> p oh ow two", two=2)[
                        :, :, kw // 2: kw // 2 + 24, kw % 2]
                    lhsT = w2T_sb[:, pos * 128:(pos + 1) * 128]
                    nc.tensor.matmul(psA[:], lhsT, vw[:, 0:12, :],
                                     start=(pos == 0), stop=(pos == 80))
                    nc.tensor.matmul(psB[:], lhsT, vw[:, 12:24, :],
                                     start=(pos == 0), stop=(pos == 80))
                # evacuate straight into group-major layout:
                # h2p3[p, g*18 + j] = h2(s = j*32 + g);  psA: j 0..8,
                # psB: j 9..17  (no bias here -- applied post-transpose)
                h2p3v = h2p3[:].rearrange("p (g j) -> p g j", j=18)
                nc.scalar.copy(h2p3v[:, :, 0:9],
                               psA[:].rearrange("p (j g) -> p g j", j=9))
                nc.scalar.copy(h2p3v[:, :, 9:18],
                               psB[:].rearrange("p (j g) -> p g j", j=9))

                # sender: a2a_in shard k (rows 128k..) = my cols for
                # groups 4k..4k+3 = h2p3[:, 72k:72k+72]
                for k in range(NC):
                    dst = bass.AP(tensor=a2a_in.tensor, offset=k * 9216,
                                  ap=[[72, 128], [1, 72]])
                    eng = nc.sync if k % 2 == 0 else nc.scalar
                    eng.dma_start(dst, h2p3[:, 72 * k:72 * (k + 1)])

            nc.gpsimd.collective_compute(
                "AllToAll", ALU.bypass,
                replica_groups=[list(range(NC))],
                ins=[a2a_in[:].opt()], outs=[a2a_out[:].opt()])

            # ============ routing phase ===================================
            with contextlib.ExitStack() as rctx:
                rp = rctx.enter_context(tc.tile_pool(name="rt", bufs=1))

                # receive: a2a_out block m=(b,occ,icc) rows t'*8+d,
                # cols gl*18+j  ->  stage[d, b*2304+gl*576+occ*288+t'*18+j]
                # (q=(b,gl) owns 576 contiguous cols in (occ,t',j) order)
                rs = tc.alloc_tile_pool(name="rs", bufs=1)
                stageA = rs.tile([8, Q * S], bf16, tag="stageA")
                stageB = rs.tile([8, Q * S], bf16, tag="stageB")
                stage8 = rs.tile([8, Q * S], bf16, tag="stage8")
                rcnt = 0
                for b in range(2):
                    for occ in range(2):
                        for icc in range(2):
                            m = 4 * b + 2 * occ + icc
                            dstt = stageB if icc else stageA
                            for gl in range(4):
                                src = bass.AP(
                                    tensor=a2a_out.tensor,
                                    offset=m * 9216 + gl * 18,
                                    ap=[[72, 8], [576, 16], [1, 18]])
                                base = b * 2304 + gl * 576 + occ * 288
                                eng = (nc.sync, nc.scalar,
                                       nc.gpsimd)[rcnt % 3]
                                rcnt += 1
                                eng.dma_start(
                                    dstt[0:8, base:base + 288], src)

                h2T = rp.tile([128, Q * PCH * D], f32, tag="h2T")
                biasT_sb = rp.tile([128, PCH * D], bf16, tag="biasT")
                tmp320 = rs.tile([128, Q * PCH * D], f32, tag="tmp320")
                sct = rs.tile([40, 128], bf16, tag="sct")
                scale_flat = rs.tile([1, 5120], bf16, tag="scale_flat")
                scale8 = rs.tile([8, 5120], bf16, tag="scale8")
                sq = rp.tile([128, 40], f32, tag="sq")
                sp1 = rp.tile([128, 40], f32, tag="sp1")
                sp2 = rp.tile([128, 40], f32, tag="sp2")
                scale = rp.tile([128, 40], f32, tag="scale")
                primT = rp.tile([8, Q * 640], bf16, tag="primT")
                ps_sc = rp.tile([128, Q * PCH * D], bf16, tag="ps_sc")
                Zt = rp.tile([128, 40], f32, tag="Z")
                rz = rp.tile([128, 40], f32, tag="rz")
                ones_sb = rp.tile([128, C], bf16, tag="ones")
                Wcf_sb = rp.tile([128, CCH * E * GL * D], bf16, tag="Wcf")
                Wcf2_sb = rp.tile([128, CCH * GL * D * E], bf16, tag="Wcf2")
                mT8 = rp.tile([8, 2 * GL * CP], f32, tag="mT8")
                m_sb = rp.tile([128, CCH * 2 * GL * D], bf16, tag="m_sb")
                s_sb = rp.tile([128, CCH * 2 * E], f32, tag="s_sb")
                sf_sb = rp.tile([128, CCH * 2 * E], f32, tag="sf_sb")
                vtmp = rp.tile([128, CCH * 2 * E], f32, tag="vtmp")
                sqv = rp.tile([128, 6], f32, tag="sqv")
                vp1 = rp.tile([128, 6], f32, tag="vp1")
                vp2 = rp.tile([128, 6], f32, tag="vp2")
                scale_v = rp.tile([128, 6], f32, tag="scale_v")
                v_sb = rp.tile([128, CCH * 2 * E], bf16, tag="v_sb")
                wv_c = rp.tile([128, CCH * 2 * GL * D], f32, tag="wv_c")
                wv_dc = rp.tile([8, 2 * GL * CP], bf16, tag="wv_dc")
                sv = rp.tile([128, 6], f32, tag="sv")
                onorm = rp.tile([128, 6], f32, tag="onorm")

                nc.scalar.dma_start(biasT_sb[:], biasT)
                nc.scalar.dma_start(Wcf_sb[:], Wcf)
                nc.scalar.dma_start(Wcf2_sb[:], Wcf2)

                nc.vector.memset(h2T[:], 0.0)
                h2Tv = h2T[:].rearrange("p (q c d) -> p q c d", q=Q, c=PCH)
                biasTv = biasT_sb[:].rearrange("p (c d) -> p c d", c=PCH)

                # icc-sum on vector (bf16 2x), then stage -> h2T
                # transposes with bias add during the PSUM->SBUF move
                with contextlib.ExitStack() as pctx:
                    ps_p = pctx.enter_context(
                        tc.tile_pool(name="psp", bufs=2, space="PSUM"))
                    ps_q = pctx.enter_context(
                        tc.tile_pool(name="psq", bufs=2, space="PSUM"))
                    for q in range(Q):
                        nc.vector.tensor_add(
                            stage8[:, q * S:(q + 1) * S],
                            stageA[:, q * S:(q + 1) * S],
                            stageB[:, q * S:(q + 1) * S])
                        tp = ps_p.tile([128, PCH * 8], bf16, tag="tacc")
                        for pch in range(PCH):
                            rows = 128 if pch < 4 else 64
                            sl = slice(q * S + pch * 128,
                                       q * S + pch * 128 + rows)
                            nc.tensor.matmul(
                                tp[0:rows, pch * 8:(pch + 1) * 8],
                                stage8[0:8, sl], ident_bf[0:8, 0:8],
                                is_transpose=True, start=True, stop=True)
                        nc.vector.tensor_add(
                            h2Tv[:, q, 0:4, :],
                            tp[:, 0:32].rearrange("p (c d) -> p c d", c=4),
                            biasTv[:, 0:4, :])
                        nc.vector.tensor_add(
                            h2Tv[0:64, q, 4, :], tp[0:64, 32:40],
                            biasTv[0:64, 4, :])

                    # squash: h2T <- h2T * sq/((1+sq)*sqrt(sq+1e-8))
                    nc.vector.tensor_mul(tmp320[:], h2T[:], h2T[:])
                    nc.vector.tensor_reduce(
                        sq[:], tmp320[:].rearrange("p (g d) -> p g d", d=D),
                        axis=AX.X, op=ALU.add)
                    nc.scalar.activation(sp1[:], sq[:], AF.Sqrt,
                                         bias=epsc[:, 0:1])
                    nc.vector.tensor_scalar_add(sp2[:], sq[:], 1.0)
                    nc.vector.tensor_mul(sp1[:], sp1[:], sp2[:])
                    nc.vector.reciprocal(sp1[:], sp1[:])
                    nc.vector.tensor_mul(scale[:], sq[:], sp1[:])
                    nc.vector.tensor_mul(
                        h2T[:].rearrange("p (g d) -> p g d", d=D),
                        h2T[:].rearrange("p (g d) -> p g d", d=D),
                        scale[:].rearrange("p (g o) -> p g o", o=1)
                        .broadcast_to([128, 40, D]))

                    # primT[d, q*640 + r] = stage8[d, q*576 + r] * scale:
                    # transpose scale once, partition-collapse via DMA,
                    # broadcast to the 8 d-rows, one vector multiply.
                    tsc = ps_q.tile([40, 128], f32, tag="tsc")
                    nc.tensor.transpose(tsc[:], scale[:], ident[:])
                    nc.scalar.copy(sct[:], tsc[:])
                    nc.sync.dma_start(scale_flat[0:1, :], sct[:])
                    for d in range(8):
                        eng = nc.sync if d % 2 == 0 else nc.scalar
                        eng.dma_start(scale8[d:d + 1, :],
                                      scale_flat[0:1, :])
                    primT_v = primT[:].rearrange("p (q r) -> p q r", r=640)
                    nc.vector.memset(primT_v[:, :, S:640], 0.0)
                    nc.vector.tensor_mul(
                        primT_v[:, :, 0:S],
                        stage8[:].rearrange("p (q r) -> p q r", q=Q),
                        scale8[:].rearrange("p (q r) -> p q r",
                                            r=640)[:, :, 0:S])

                rs.release()

                ps_t = rctx.enter_context(
                    tc.tile_pool(name="pst", bufs=1, space="PSUM"))
                ps_m = rctx.enter_context(
                    tc.tile_pool(name="psm", bufs=2, space="PSUM"))
                ps_u = rctx.enter_context(
                    tc.tile_pool(name="psu", bufs=5, space="PSUM"))

                nc.vector.memset(ones_sb[:], 1.0)
                nc.vector.memset(mT8[:], 0.0)

                rl = rctx.enter_context(tc.tile_pool(name="rl", bufs=1))
                e1 = rl.tile([128, Q * PCH * C], bf16, tag="e1")
                eup = rctx.enter_context(tc.tile_pool(name="eup", bufs=2))

                e_lv = e1[:].rearrange("p (q c x) -> p q c x", q=Q, c=PCH)
                psc = ps_sc[:].rearrange("p (g d) -> p g d", d=D)
                Wcfv = Wcf_sb[:].rearrange("p (c e g d) -> p c e g d",
                                           c=CCH, e=E, g=GL)
                Wcf2v = Wcf2_sb[:].rearrange("p (c g d e) -> p c g d e",
                                             c=CCH, g=GL, d=D)
                m_v = m_sb[:].rearrange("p (c b x) -> p c b x", c=CCH, b=2)
                s_v = s_sb[:].rearrange("p (c b e) -> p c b e", c=CCH, b=2)
                sf_v = sf_sb[:].rearrange("p (c b e) -> p c b e", c=CCH, b=2)
                v_v = v_sb[:].rearrange("p (c b e) -> p c b e", c=CCH, b=2)
                wv_v = wv_c[:].rearrange("p (c b x) -> p c b x", c=CCH, b=2)

                h2Tgd = h2T[:].rearrange("p (g d) -> p g d", d=D)
                up_tiles = None
                for it in range(3):
                    if it >= 1:
                        for _w in range(24):
                            wps = ps_t.tile([128, 8], f32, tag="tmix")
                            nc.tensor.transpose(
                                wps[:], ident[0:8, :], ident[0:8, 0:8])
                    # --- coupling coefficients -> scaled prim rows, then
                    # m^T[d, c] per row-block; per-q software pipeline
                    for q in range(Q):
                        if it == 0:
                            if q == 0:
                                nc.scalar.mul(ps_sc[:], h2T[:], 1.0 / C)
                        else:
                            if it == 1:
                                # e1 = exp(u1), straight from PSUM
                                for pch in range(PCH):
                                    nc.scalar.activation(
                                        e_lv[:, q, pch, :],
                                        up_tiles[q][pch][:, 0:C],
                                        AF.Exp, bias=zeroc[:, 0:1])
                            else:
                                # e2 = e1 * exp(u2)
                                eu = eup.tile([128, PCH * C], bf16, tag="eu")
                                euv = eu[:].rearrange("p (c x) -> p c x",
                                                      c=PCH)
                                for pch in range(PCH):
                                    nc.scalar.activation(
                                        euv[:, pch, :],
                                        up_tiles[q][pch][:, 0:C],
                                        AF.Exp, bias=zeroc[:, 0:1])
                                eq = e_lv[:, q, :, :].rearrange(
                                    "p c x -> p (c x)")
                                nc.vector.tensor_mul(eq, eq, eu[:])
                            nc.vector.tensor_reduce(
                                Zt[:, q * PCH:(q + 1) * PCH],
                                e_lv[:, q, :, :], axis=AX.X, op=ALU.add)
                            nc.vector.reciprocal(
                                rz[:, q * PCH:(q + 1) * PCH],
                                Zt[:, q * PCH:(q + 1) * PCH])
                            nc.vector.tensor_mul(
                                psc[:, q * PCH:(q + 1) * PCH, :],
                                h2Tgd[:, q * PCH:(q + 1) * PCH, :],
                                rz[:, q * PCH:(q + 1) * PCH]
                                .rearrange("p (g o) -> p g o", o=1)
                                .broadcast_to([128, PCH, D]))
                        mp = ps_m.tile([8, C], f32, tag="mT")
                        for pch in range(PCH):
                            rhs = (ones_sb[:] if it == 0
                                   else e_lv[:, q, pch, :])
                            nc.tensor.matmul(mp[:],
                                             psc[:, q * PCH + pch, :],
                                             rhs, start=(pch == 0),
                                             stop=(pch == PCH - 1))
                        nc.scalar.copy(mT8[0:8, q * CP:q * CP + C], mp[:])

                    # --- transpose m to class-partitioned layout + votes
                    for b in range(2):
                        for ch in range(CCH):
                            tp = ps_t.tile([128, 32], f32, tag="tmix")
                            for g in range(GL):
                                q = b * GL + g
                                nc.tensor.transpose(
                                    tp[:, g * D:(g + 1) * D],
                                    mT8[0:8, q * CP + ch * 128:
                                        q * CP + (ch + 1) * 128],
                                    ident[0:8, 0:8])
                            nc.vector.tensor_copy(m_v[:, ch, b, :], tp[:])

                    # --- class votes s[c, e] = sum_{g,d} m * W (gpsimd)
                    for b in range(2):
                        for ch in range(CCH):
                            st = rp.tile([128, E * GL * D], bf16, tag="stmp")
                            nc.gpsimd.tensor_mul(
                                st[:].rearrange("p (e g d) -> p e g d",
                                                e=E, g=GL),
                                Wcfv[:, ch],
                                m_v[:, ch, b, :]
                                .rearrange("p (o g d) -> p o g d", o=1, g=GL)
                                .broadcast_to([128, E, GL, D]))
                            nc.vector.tensor_reduce(
                                s_v[:, ch, b, :],
                                st[:].rearrange("p (e x) -> p e x", e=E),
                                axis=AX.X, op=ALU.add)

                    if it < 2:
                        for _w in range(64):
                            wps = ps_t.tile([128, 8], f32, tag="tmix")
                            nc.tensor.transpose(
                                wps[:], ident[0:8, :], ident[0:8, 0:8])
                    nc.sync.dma_start(s_ins[it][:], s_sb[:])
                    nc.gpsimd.collective_compute(
                        "AllReduce", ALU.add,
                        replica_groups=[list(range(NC))],
                        ins=[s_ins[it][:].opt()],
                        outs=[s_outs[it][:].opt()])
                    nc.sync.dma_start(sf_sb[:], s_outs[it][:])

                    # --- v = squash(s) pieces
                    nc.vector.tensor_mul(vtmp[:], sf_sb[:], sf_sb[:])
                    nc.vector.tensor_reduce(
                        sqv[:], vtmp[:].rearrange("p (g e) -> p g e", e=E),
                        axis=AX.X, op=ALU.add)
                    nc.scalar.activation(vp1[:], sqv[:], AF.Sqrt,
                                         bias=epsc[:, 0:1])
                    nc.vector.tensor_scalar_add(vp2[:], sqv[:], 1.0)
                    nc.vector.tensor_mul(vp1[:], vp1[:], vp2[:])
                    nc.vector.reciprocal(vp1[:], vp1[:])
                    nc.vector.tensor_mul(scale_v[:], sqv[:], vp1[:])

                    if it < 2:
                        nc.vector.tensor_mul(
                            v_sb[:].rearrange("p (g e) -> p g e", e=E),
                            sf_sb[:].rearrange("p (g e) -> p g e", e=E),
                            scale_v[:].rearrange("p (g o) -> p g o", o=1)
                            .broadcast_to([128, 6, E]))
                        # wv[c, (g,d)] = sum_e W2 * v
                        for b in range(2):
                            for ch in range(CCH):
                                wt = rp.tile([128, GL * D * E], bf16,
                                             tag="wtmp")
                                nc.gpsimd.tensor_mul(
                                    wt[:].rearrange(
                                        "p (g d e) -> p g d e", g=GL, d=D),
                                    Wcf2v[:, ch],
                                    v_v[:, ch, b, :]
                                    .rearrange("p (o u e) -> p o u e",
                                               o=1, u=1)
                                    .broadcast_to([128, GL, D, E]))
                                nc.vector.tensor_reduce(
                                    wv_v[:, ch, b, :],
                                    wt[:].rearrange("p (x e) -> p x e", e=E),
                                    axis=AX.X, op=ALU.add)
                        # transpose wv to [d, c]
                        for b in range(2):
                            for ch in range(CCH):
                                tp = ps_t.tile([8, 512], f32, tag="tmix")
                                for g in range(GL):
                                    nc.tensor.transpose(
                                        tp[:, g * 128:(g + 1) * 128],
                                        wv_v[:, ch, b, g * D:(g + 1) * D],
                                        ident[:])
                                dst = wv_dc[0:8, :].rearrange(
                                    "p (q c x) -> p q c x", q=Q, c=CCH)[
                                    :, b * GL:(b + 1) * GL, ch, :]
                                nc.scalar.copy(dst, tp[:])
                        # u += prim @ wv, kept in PSUM; next iteration
                        # exponentiates it directly (e2 = e1 * exp(u2))
                        up_tiles = []
                        for q in range(Q):
                            row = []
                            for pch in range(PCH):
                                up = ps_u.tile([128, 512], f32, tag="u")
                                nc.tensor.matmul(
                                    up[:, 0:C],
                                    primT[0:8, q * 640 + pch * 128:
                                          q * 640 + (pch + 1) * 128],
                                    wv_dc[0:8, q * CP:q * CP + C],
                                    start=True, stop=True)
                                row.append(up)
                            up_tiles.append(row)
                    else:
                        # output ||v|| = sqrt(sqv) * scale_v
                        nc.scalar.activation(sv[:], sqv[:], AF.Sqrt,
                                             bias=zeroc[:, 0:1])
                        nc.vector.tensor_mul(onorm[:], sv[:], scale_v[:])
                        ov = onorm[:].rearrange("p (g b) -> p g b", b=2)
                        for b in range(2):
                            nc.sync.dma_start(
                                out[b:b + 1, 0:256].rearrange(
                                    "o (ch p) -> o p ch", p=128),
                                ov[:, 0:2, b])
                            nc.sync.dma_start(out[b:b + 1, 256:276],
                                              ov[0:20, 2, b])

    nc.compile()
    return nc


def _host_prep(x, conv1_w, conv1_b, conv2_w, conv2_b, W):
    """Build the 8 per-core input maps."""
    x = np.asarray(x, np.float32)
    w1T_full = np.asarray(conv1_w, np.float32).reshape(256, 81).T.copy()
    w2 = np.asarray(conv2_w, np.float32).reshape(256, 256, 81)
    # [ic, pos, oc]
    w2T_full = np.ascontiguousarray(w2.transpose(1, 2, 0))
    conv1_b = np.asarray(conv1_b, np.float32)
    conv2_b = np.asarray(conv2_b, np.float32)
    W = np.asarray(W, np.float32)  # [32, 276, 8, 16]

    # biasT[r, pch*8+d] = conv2_b[(occ*16+t')*8+d] for
    # p_local = pch*128+r = occ*288+t'*18+j  (rows >= 576 zero)
    biasT = np.zeros((128, PCH * D), BF16)
    for pl in range(S):
        occ, rem = divmod(pl, 288)
        t_, _j = divmod(rem, 18)
        pch, r = divmod(pl, 128)
        biasT[r, pch * 8:(pch + 1) * 8] = \
            conv2_b[(occ * 16 + t_) * 8:(occ * 16 + t_) * 8 + 8]

    in_maps = []
    for k in range(NC):
        b_k, occ_k, icc_k = k >> 2, (k >> 1) & 1, k & 1
        xb = np.ascontiguousarray(x[b_k, 0]).astype(BF16)
        w1T = np.ascontiguousarray(
            w1T_full[:, icc_k * 128:(icc_k + 1) * 128]).astype(BF16)
        b1 = conv1_b[icc_k * 128:(icc_k + 1) * 128].reshape(128, 1).copy()
        w2T = np.ascontiguousarray(
            w2T_full[icc_k * 128:(icc_k + 1) * 128, :,
                     occ_k * 128:(occ_k + 1) * 128]).reshape(
                         128, 81 * 128).astype(BF16)

        # Wcf[p, ch, e, g_l, d] / Wcf2[p, ch, g_l, d, e] = W[4k+g_l, c, d, e]
        Wk = W[4 * k:4 * k + 4]  # [GL, 276, 8, 16]
        Wp = np.zeros((GL, CP, D, E), np.float32)
        Wp[:, :C] = Wk
        Wp = Wp.reshape(GL, CCH, 128, D, E)
        Wcf = np.ascontiguousarray(
            Wp.transpose(2, 1, 4, 0, 3)).reshape(128, -1).astype(BF16)
        Wcf2 = np.ascontiguousarray(
            Wp.transpose(2, 1, 0, 3, 4)).reshape(128, -1).astype(BF16)

        in_maps.append({
            "xb": xb, "w1T": w1T, "b1": b1, "w2T": w2T,
            "biasT": biasT, "Wcf": Wcf, "Wcf2": Wcf2,
        })
    return in_maps


def kernel(x, conv1_w, conv1_b, conv2_w, conv2_b, W):
    if "nc" not in _CACHE:
        _CACHE["nc"] = _build_program()
    nc = _CACHE["nc"]
    in_maps = _host_prep(x, conv1_w, conv1_b, conv2_w, conv2_b, W)

    from concourse.bass_utils import run_bass_kernel_spmd
    res = run_bass_kernel_spmd(nc, in_maps, core_ids=list(range(NC)),
                               trace=bool(int(os.environ.get(
                                   "CAPS_TRACE", "0"))))
    _CACHE["last_result"] = res
    return np.asarray(res.results[0]["out"], np.float32)
